# revision 20
# baseline (speedup 1.0000x reference)
"""Multi-Head Latent Attention (MLA) Bass kernel for 8 Trainium2 NeuronCores.

Sharding (v5):
  - latent projections (d_kv, d_q): sequence-sharded (BS/8 rows per core),
    fp16 AllGathers replicate the latents.
  - up-projections (u_k, u_v, u_q, qr) + attention: head-sharded, 2 heads/core.
  - context: AllToAll re-shards to sequence; out_proj sequence-parallel.

Precision strategy (error budget is the binding constraint; measured fp8e4m3
rms rel err is 5.8% and softmax amplifies score noise ~3.5x into the output,
so each fp8 stage costs ~1.5-2.4% of output rel err vs the 2e-2 gate):
  - Exactly ONE fp8 stage: the stored k8/q8 score operands.  The S*S score
    matmuls run fp8 MatmulPerfMode.DoubleRow at 0.5 cycles/row (4x bf16).
    Layout per head: [128 partitions, 2 k-tiles, BS]; tile0 = k_c dims
    0..127, tile1 = [rope 0..63 | zeros].  The dead zero rows are free --
    DoubleRow cost depends only on the moving free size.  Zero-padding is
    memset once (fp8 garbage could be NaN and NaN*0 propagates).
  - Everything else (latents, all projection weights, V, probs, out-proj)
    is fp16: same PE speed as bf16, 8x less quantization noise.
  - The softmax scale rides the exp activation's `scale` operand.
  - exp is batched 2 sk-tiles per activation ([128, 2048]); the Activation
    engine is the attention-phase bottleneck (~15.1us/block), PE runs under.
  - rope/V biases are zeros by problem spec and not injected on device where
    they would cost extra instructions (qr_b, u_v_b); out_b is added on host.
    d_kv_b/d_q_b/u_k_b/u_q_b ride along free on the copy path.
  - woT (out-proj weights) prefetch into SBUF during attention so phase D
    starts immediately after the AllToAll.  Write-only filler matmuls keep
    the PE clock (HAM p-state) warm across the two exposed collective waits.
"""
import sys
import os

for _p in ("/opt/trn_rl_repo", "/root/.axon_site/_ro/trn_rl_repo"):
    if os.path.isdir(_p) and _p not in sys.path:
        sys.path.insert(0, _p)

import math
import numpy as np
import ml_dtypes

F8NP = ml_dtypes.float8_e4m3

import concourse.bacc as bacc
import concourse.mybir as mybir
from concourse import tile
from concourse.bass_utils import run_bass_kernel_spmd

# problem dims (hardcoded)
B, S, H, Dh, Dr, HID, C = 2, 2048, 16, 128, 64, 2048, 512
BS = B * S                      # 4096
NCORES = 8
H_LOC = H // NCORES             # 2
S_LOC = BS // NCORES            # 512
SCALE = 1.0 / math.sqrt(Dh + Dr)

F32 = mybir.dt.float32
F16 = mybir.dt.float16
F8 = mybir.dt.float8e4
DR = mybir.MatmulPerfMode.DoubleRow

_CACHE = {}


def _build_program():
    nc = bacc.Bacc("TRN2", target_bir_lowering=False, debug=False,
                   num_devices=NCORES)

    # ---- phase A inputs ----
    xt_in = nc.dram_tensor("xt_in", [4, 128, 4, S_LOC], F16, kind="ExternalInput")
    wkv_in = nc.dram_tensor("wkv_in", [4, 128, 4, C], F16, kind="ExternalInput")
    wdq_in = nc.dram_tensor("wdq_in", [4, 128, 4, C], F16, kind="ExternalInput")
    bdkv = nc.dram_tensor("bdkv", [4, 128, 1], F32, kind="ExternalInput")
    bdq = nc.dram_tensor("bdq", [4, 128, 1], F32, kind="ExternalInput")
    # ---- up-projection weights (fp16) ----
    ukT = nc.dram_tensor("ukT", [C, 256], F16, kind="ExternalInput")
    uqT = nc.dram_tensor("uqT", [C, 256], F16, kind="ExternalInput")
    uvT = nc.dram_tensor("uvT", [C, 256], F16, kind="ExternalInput")
    wraT = nc.dram_tensor("wraT", [C, 128], F16, kind="ExternalInput")
    wrbT = nc.dram_tensor("wrbT", [C, 128], F16, kind="ExternalInput")
    buk = nc.dram_tensor("buk", [2, 128, 1], F32, kind="ExternalInput")
    buq = nc.dram_tensor("buq", [2, 128, 1], F32, kind="ExternalInput")
    c1_in = nc.dram_tensor("c1", [128, S], F16, kind="ExternalInput")
    c2_in = nc.dram_tensor("c2", [128, S], F16, kind="ExternalInput")

    woT = nc.dram_tensor("woT", [H * Dh, HID], F16, kind="ExternalInput")
    out_sl = nc.dram_tensor("out_slice", [S_LOC, HID], F16, kind="ExternalOutput")

    groups = [list(range(NCORES))]
    EXP = mybir.ActivationFunctionType.Exp
    IDENT = mybir.ActivationFunctionType.Identity

    with tile.TileContext(nc) as tc:
        with tc.tile_pool(name="dram", bufs=1, space="DRAM") as dram:
            kv16_i = dram.tile([4, 128, S_LOC], F16)
            kv16_o = dram.tile([NCORES, 4, 128, S_LOC], F16,
                               addr_space="Shared", name="kv16o")
            ql16_i = dram.tile([4, 128, S_LOC], F16)
            ql16_o = dram.tile([NCORES, 4, 128, S_LOC], F16,
                               addr_space="Shared", name="ql16o")
            a2a_i = dram.tile([NCORES, H_LOC * Dh, S_LOC], F16)
            a2a_o = dram.tile([NCORES, H_LOC * Dh, S_LOC], F16, name="a2ao")

            with tc.tile_pool(name="const", bufs=1) as const:
                ones_col = const.tile([128, 1], F16)
                nc.vector.memset(ones_col[:], 1.0)

                # ====== persistent attention operand tiles ======
                with tc.tile_pool(name="attn", bufs=1) as attn:
                    # [128 part, 2 k-tiles, BS] fp8 DoubleRow score operands
                    k8 = [attn.tile([128, 2, BS], F8, tag=f"k8_{h}",
                                    name=f"k8_{h}") for h in range(2)]
                    q8 = [attn.tile([128, 2, BS], F8, tag=f"q8_{h}",
                                    name=f"q8_{h}") for h in range(2)]
                    # zero the dead rows of k-tile 1 once (fp8 garbage could
                    # be NaN, and NaN*0 propagates through the PE)
                    for t_ in (k8[0], k8[1], q8[0], q8[1]):
                        nc.vector.memset(t_[64:128, 1, :], 0.0)
                    v_sb = attn.tile([128, 32, 256], F16, tag="v", name="v_sb")
                    c1s = attn.tile([128, S], F16, tag="c1", name="c1s")
                    nc.sync.dma_start(out=c1s[:], in_=c1_in[:])
                    c2s = attn.tile([128, S], F16, tag="c2", name="c2s")
                    nc.sync.dma_start(out=c2s[:], in_=c2_in[:])

                    # resident operand for HAM-warming filler matmuls
                    junk8 = attn.tile([128, 2, S_LOC], F8, tag="junk8",
                                      name="junk8")
                    nc.vector.memset(junk8[:], 0.25)

                    # ============ Phase A: latents (sequence-sharded) ========
                    with tc.tile_pool(name="phA", bufs=1) as phA, \
                         tc.tile_pool(name="phAb", bufs=4) as phAb, \
                         tc.tile_pool(name="psA", bufs=2, space="PSUM") as psA:
                        engs = (nc.sync, nc.gpsimd, nc.scalar)
                        ei = 0
                        xts, wkv, wdq = [], [], []
                        for q4 in range(4):
                            xt = phA.tile([128, 4, S_LOC], F16, tag=f"xt{q4}",
                                          name=f"xt{q4}")
                            for i2 in range(2):
                                engs[ei % 3].dma_start(
                                    out=xt[:, 2 * i2:2 * i2 + 2, :],
                                    in_=xt_in[q4, :, 2 * i2:2 * i2 + 2, :])
                                ei += 1
                            xts.append(xt)
                            w = phA.tile([128, 4, C], F16, tag=f"wk{q4}",
                                         name=f"wk{q4}")
                            for i2 in range(2):
                                engs[ei % 3].dma_start(
                                    out=w[:, 2 * i2:2 * i2 + 2, :],
                                    in_=wkv_in[q4, :, 2 * i2:2 * i2 + 2, :])
                                ei += 1
                            wkv.append(w)
                        for q4 in range(4):
                            w = phA.tile([128, 4, C], F16, tag=f"wq{q4}",
                                         name=f"wq{q4}")
                            for i2 in range(2):
                                engs[ei % 3].dma_start(
                                    out=w[:, 2 * i2:2 * i2 + 2, :],
                                    in_=wdq_in[q4, :, 2 * i2:2 * i2 + 2, :])
                                ei += 1
                            wdq.append(w)

                        # ramp the PE clock while the startup DMAs land
                        psw = psA.tile([128, S_LOC], F32, tag="warm",
                                       name="psw")
                        for _ in range(60):
                            nc.tensor.matmul(psw[:], junk8[:, :, 0:128],
                                             junk8[:], start=True, stop=True,
                                             perf_mode=DR)
                        for li, (wfull, bdram, agi, ago) in enumerate(
                                ((wkv, bdkv, kv16_i, kv16_o),
                                 (wdq, bdq, ql16_i, ql16_o))):
                            for ct in range(4):
                                ps = psA.tile([128, S_LOC], F32, tag="ps",
                                              name="psa")
                                for ht in range(16):
                                    nc.tensor.matmul(
                                        ps[:],
                                        wfull[ht // 4][:, ht % 4,
                                                       ct * 128:(ct + 1) * 128],
                                        xts[ht // 4][:, ht % 4, :],
                                        start=(ht == 0), stop=(ht == 15))
                                bt = phAb.tile([128, 1], F32, tag="blat",
                                               name="blat")
                                nc.scalar.dma_start(out=bt[:], in_=bdram[ct])
                                l16 = phAb.tile([128, S_LOC], F16, tag="l16",
                                                bufs=3, name="l16")
                                nc.scalar.activation(l16[:], ps[:], IDENT,
                                                     bias=bt[:])
                                nc.sync.dma_start(out=agi[ct], in_=l16[:])
                            nc.gpsimd.collective_compute(
                                "AllGather", mybir.AluOpType.bypass,
                                replica_groups=groups,
                                ins=[agi.opt()], outs=[ago.opt()])

                    # persistent up-projection weights
                    upw = {}
                    for nm, t, w_ in (("uk", ukT, 256), ("uq", uqT, 256),
                                      ("uv", uvT, 256),
                                      ("wra", wraT, 128), ("wrb", wrbT, 128)):
                        tl = []
                        for ct in range(4):
                            wt = attn.tile([128, w_], F16, tag=f"{nm}{ct}",
                                           name=f"{nm}{ct}")
                            nc.sync.dma_start(
                                out=wt[:], in_=t[ct * 128:(ct + 1) * 128, :])
                            tl.append(wt)
                        upw[nm] = tl
                    bias_t = {}
                    for nm, t in (("buk", buk), ("buq", buq)):
                        tl = []
                        for h in range(2):
                            bt_ = attn.tile([128, 1], F32, tag=f"{nm}{h}",
                                            name=f"{nm}{h}")
                            nc.scalar.dma_start(out=bt_[:], in_=t[h])
                            tl.append(bt_)
                        bias_t[nm] = tl
                    # woT prefetch: wo_sb[:, dht, ot, :] = woT[dht*128+p,
                    # ot*512+c] for ot 0..1; second column-half streams in
                    # phase D.  Half loads now, half during attention.
                    wo_sb = attn.tile([128, 16, 2, S_LOC], F16, tag="wo",
                                      name="wo_sb")

                    # keep the PE clock un-throttled across the AG-kv wait:
                    # write-only filler matmuls on resident tiles.
                    with tc.tile_pool(name="psj", bufs=1, space="PSUM") as psjp:
                        psj = psjp.tile([128, S_LOC], F32, tag="junk",
                                        name="psj")
                        for _ in range(140):
                            nc.tensor.matmul(psj[:], junk8[:, :, 0:128],
                                             junk8[:], start=True, stop=True,
                                             perf_mode=DR)

                    # ========= phase B: kv-dependent (k_c, rope-k, V) ========
                    def up_project(j2, w_lo, b_lo, dst, src, phX, psX):
                        """fp16 up-projection of one 512-col block into the
                        fp8 [128, 2, BS] DoubleRow score layout."""
                        sl = slice(j2 * 512, (j2 + 1) * 512)
                        pos = slice((j2 % 4) * 512, (j2 % 4) * 512 + 512)
                        for h in range(2):
                            psK = psX.tile([128, 512], F32, tag="psK",
                                           bufs=2, name="psk")
                            hc = slice(h * 128, (h + 1) * 128)
                            for ct in range(4):
                                nc.tensor.matmul(psK[:],
                                                 w_lo[ct][:, hc],
                                                 src[ct][:],
                                                 start=(ct == 0),
                                                 stop=(ct == 3))
                            nc.scalar.activation(dst[h][:, 0, sl],
                                                 psK[:], IDENT,
                                                 bias=b_lo[h][:])
                        # rope for both heads stacked on 128 partitions
                        psRA = psX.tile([128, 512], F32, tag="psRA", bufs=1,
                                        name="psra")
                        psRB = psX.tile([128, 512], F32, tag="psRB", bufs=1,
                                        name="psrb")
                        for ct in range(4):
                            nc.tensor.matmul(psRA[:], upw["wra"][ct][:],
                                             src[ct][:],
                                             start=(ct == 0), stop=(ct == 3))
                        for ct in range(4):
                            nc.tensor.matmul(psRB[:], upw["wrb"][ct][:],
                                             src[ct][:],
                                             start=(ct == 0), stop=(ct == 3))
                        t1 = phX.tile([128, 512], F16, tag="t1", bufs=2,
                                      name="t1")
                        nc.vector.tensor_mul(t1[:], psRA[:], c1s[:, pos])
                        t2 = phX.tile([128, 512], F16, tag="t2", bufs=2,
                                      name="t2")
                        nc.vector.tensor_mul(t2[:], psRB[:], c2s[:, pos])
                        stgr = phX.tile([128, 512], F8, tag="stgr", bufs=2,
                                        name="stgr")
                        nc.vector.tensor_add(stgr[:], t1[:], t2[:])
                        # partition-shift the per-head rope halves into place
                        nc.sync.dma_start(out=dst[0][0:64, 1, sl],
                                          in_=stgr[0:64, :])
                        nc.sync.dma_start(out=dst[1][0:64, 1, sl],
                                          in_=stgr[64:128, :])

                    with tc.tile_pool(name="phB", bufs=1) as phB, \
                         tc.tile_pool(name="psB", bufs=1, space="PSUM") as psB:
                        for j2 in range(8):
                            kv_sb = []
                            for ct in range(4):
                                kt_ = phB.tile([128, 512], F16,
                                               tag=f"kv_{ct}", bufs=2,
                                               name=f"kv_{ct}")
                                eng = (nc.scalar, nc.gpsimd, nc.sync,
                                       nc.scalar)[ct]
                                eng.dma_start(out=kt_[:], in_=kv16_o[j2, ct])
                                kv_sb.append(kt_)
                            up_project(j2, upw["uk"], bias_t["buk"],
                                       k8, kv_sb, phB, psB)
                            # V projection (fp16); u_v_b is zero by problem
                            # spec (bias varies along the free dim here, so
                            # not fusable into the copy)
                            for ss in range(4):
                                psv_ = psB.tile([128, 256], F32, tag="psV",
                                                bufs=2, name="psv")
                                ssl = slice(ss * 128, (ss + 1) * 128)
                                for ct in range(4):
                                    nc.tensor.matmul(psv_[:],
                                                     kv_sb[ct][:, ssl],
                                                     upw["uv"][ct][:],
                                                     start=(ct == 0),
                                                     stop=(ct == 3))
                                st = j2 * 4 + ss
                                nc.vector.tensor_copy(v_sb[:, st, :], psv_[:])

                    # preload the gpsimd extended-instruction lib before
                    # attention so the first partition_broadcast doesn't pay
                    # the LOAD_LIB swap mid-attention.
                    pbin = const.tile([1, 64], F32)
                    nc.vector.memset(pbin[:], 1.0)
                    pbout = const.tile([128, 64], F32)
                    nc.gpsimd.partition_broadcast(pbout[:], pbin[:])

                    # ================= pass 2: q-side projections ============
                    with tc.tile_pool(name="phQ", bufs=1) as phQ, \
                         tc.tile_pool(name="psQ", bufs=1, space="PSUM") as psQ:
                        for g in range(8):
                            ql_sb = []
                            for ct in range(4):
                                qt_ = phQ.tile([128, 512], F16,
                                               tag=f"ql_{ct}", bufs=2,
                                               name=f"ql_{ct}")
                                eng = (nc.scalar, nc.gpsimd, nc.sync,
                                       nc.scalar)[ct]
                                eng.dma_start(out=qt_[:], in_=ql16_o[g, ct])
                                ql_sb.append(qt_)
                            up_project(g, upw["uq"], bias_t["buq"],
                                       q8, ql_sb, phQ, psQ)
                        # first half of woT while attention operands settle
                        for dht in range(8):
                            nc.gpsimd.dma_start(
                                out=wo_sb[:, dht, :, :],
                                in_=woT[dht * 128:(dht + 1) * 128, 0:1024])

                    # ================= Phase C: attention ====================
                    with tc.tile_pool(name="phC", bufs=1) as phC, \
                         tc.tile_pool(name="psC", bufs=1, space="PSUM") as psC:
                        psvs = [psC.tile([128, 512], F32, tag=f"psv{h}",
                                         bufs=1, name=f"psv{h}")
                                for h in range(2)]

                        def denom_start(probs_p):
                            """fold 16 sk-tiles -> 4 on DVE (fp16, 2x mode)."""
                            p8 = phC.tile([128, 8, 2, 512], F16, tag="p8",
                                          bufs=1, name="p8")
                            nc.vector.tensor_add(p8[:],
                                                 probs_p[:, 0:8, :, :],
                                                 probs_p[:, 8:16, :, :])
                            nc.vector.tensor_add(p8[:, 0:4], p8[:, 0:4],
                                                 p8[:, 4:8])
                            return p8

                        def colsum(p8, dsum, i):
                            for h in range(2):
                                nc.tensor.matmul(dsum[0:1, h, :], ones_col[:],
                                                 p8[:, i, h, :],
                                                 start=(i == 0), stop=(i == 3))

                        def recip_bcast(dsum):
                            rec = phC.tile([1, 2, 512], F32, tag="rec",
                                           bufs=1, name="rec")
                            recb = phC.tile([128, 2, 512], F32, tag="recb",
                                            bufs=1, name="recb")
                            nc.vector.reciprocal_approx_accurate(
                                out=rec[0:1, :, :], in_=dsum[0:1, :, :],
                                scratch=recb[0:1, :, :])
                            nc.gpsimd.partition_broadcast(recb[:],
                                                          rec[0:1, :, :])
                            return recb

                        def finish_stg(pg, recb):
                            for h in range(2):
                                stg = phC.tile([128, 512], F16, tag=f"stg{h}",
                                               bufs=1, name=f"stg{h}")
                                nc.vector.tensor_mul(stg[:], psvs[h][:],
                                                     recb[:, h, :])
                                nc.sync.dma_start(
                                    out=a2a_i[pg, h * 128:(h + 1) * 128, :],
                                    in_=stg[:])

                        # software pipeline: block g's score/exp stream carries
                        # block g-1's PV, denominator chain and normalization.
                        prev = None
                        for g in range(8):
                            b, sqb = g // 4, g % 4
                            qsl = slice(b * S + sqb * 512,
                                        b * S + sqb * 512 + 512)
                            probs = phC.tile([128, 16, 2, 512], F16,
                                             tag="probs", bufs=2, name="probs")
                            if prev is not None:
                                pg, pb, probs_p = prev
                                p8 = denom_start(probs_p)
                                dsum = psC.tile([1, 2, 512], F32, tag="dsum",
                                                bufs=1, name="dsum")
                                recb = None
                                pv_idx = 0
                            for skt in range(16):
                                psb_ = psC.tile([128, 2, 512], F32,
                                                tag="ps2", bufs=2, name="ps2")
                                ksl = slice(b * S + skt * 128,
                                            b * S + skt * 128 + 128)
                                for _ in range(2):
                                    nc.tensor.matmul(psb_[:, 0, :],
                                                     junk8[:, :, 0:128],
                                                     junk8[:],
                                                     start=True, stop=True,
                                                     perf_mode=DR)
                                for h in range(2):
                                    nc.tensor.matmul(
                                        psb_[:, h, :],
                                        k8[h][:, :, ksl],
                                        q8[h][:, :, qsl],
                                        start=True, stop=True,
                                        perf_mode=DR)
                                nc.scalar.activation(
                                    probs[:, skt, :, :],
                                    psb_[:], EXP, scale=SCALE)
                                if prev is not None:
                                    # prev-block PV: 16 sk-tiles x 2 heads,
                                    # ~2-3 matmuls per score group
                                    while (pv_idx < 32 and
                                           pv_idx * 14 // 32 <= skt and
                                           skt < 14):
                                        sktp = pv_idx // 2
                                        h = pv_idx % 2
                                        nc.tensor.matmul(
                                            psvs[h][:],
                                            v_sb[:, pb * 16 + sktp,
                                                 h * 128:(h + 1) * 128],
                                            probs_p[:, sktp, h, :],
                                            start=(sktp == 0),
                                            stop=(sktp == 15))
                                        pv_idx += 1
                                    if skt in (7, 9, 11, 13):
                                        colsum(p8, dsum, (skt - 7) // 2)
                                    if skt == 13:
                                        recb = recip_bcast(dsum)
                            if prev is not None:
                                finish_stg(prev[0], recb)
                            # second half of woT, spread across blocks
                            nc.gpsimd.dma_start(
                                out=wo_sb[:, 8 + g, :, :],
                                in_=woT[(8 + g) * 128:(9 + g) * 128, 0:1024])
                            prev = (g, b, probs)
                        # drain the last block
                        pg, pb, probs_p = prev
                        p8 = denom_start(probs_p)
                        dsum = psC.tile([1, 2, 512], F32, tag="dsum",
                                        bufs=1, name="dsum")
                        for sktp in range(16):
                            for h in range(2):
                                nc.tensor.matmul(
                                    psvs[h][:],
                                    v_sb[:, pb * 16 + sktp,
                                         h * 128:(h + 1) * 128],
                                    probs_p[:, sktp, h, :],
                                    start=(sktp == 0), stop=(sktp == 15))
                        for i in range(4):
                            colsum(p8, dsum, i)
                        finish_stg(pg, recip_bcast(dsum))
                        nc.gpsimd.collective_compute(
                            "AllToAll", mybir.AluOpType.bypass,
                            replica_groups=groups,
                            ins=[a2a_i.opt()], outs=[a2a_o.opt()])

                    # ============== Phase D: out projection ==================
                    with tc.tile_pool(name="phD", bufs=1) as phD, \
                         tc.tile_pool(name="phDw", bufs=2) as phDw, \
                         tc.tile_pool(name="phDo", bufs=3) as phDo, \
                         tc.tile_pool(name="psD", bufs=2, space="PSUM") as psD:
                        # ot=2,3 weight columns load during the A2A wait
                        wos_hi = []
                        for dht in range(16):
                            wo = phDw.tile([128, 2, 512], F16,
                                           tag=f"wo{dht}", bufs=1,
                                           name=f"wo{dht}")
                            eng = (nc.sync, nc.gpsimd, nc.scalar)[dht % 3]
                            eng.dma_start(
                                out=wo[:],
                                in_=woT[dht * 128:(dht + 1) * 128,
                                        1024:2048])
                            wos_hi.append(wo)
                        # filler over the AllToAll wait
                        psj2 = psD.tile([128, 512], F32, tag="junk",
                                        bufs=1, name="psj2")
                        for _ in range(110):
                            nc.tensor.matmul(psj2[:], junk8[:, :, 0:128],
                                             junk8[:], start=True, stop=True,
                                             perf_mode=DR)
                        csl = []
                        for dht in range(16):
                            cf = phD.tile([128, S_LOC], F16,
                                          tag=f"cf{dht}", name=f"cf{dht}")
                            eng = nc.sync if dht % 2 == 0 else nc.gpsimd
                            eng.dma_start(
                                out=cf[:],
                                in_=a2a_o[dht // 2,
                                          (dht % 2) * 128:
                                          (dht % 2) * 128 + 128, :])
                            csl.append(cf)
                        for ot in range(4):
                            osl = slice(ot * 512, (ot + 1) * 512)
                            if ot < 2:
                                wos = [wo_sb[:, dht, ot, :]
                                       for dht in range(16)]
                            else:
                                wos = [wos_hi[dht][:, ot - 2, :]
                                       for dht in range(16)]
                            for ssub in range(4):
                                pso = psD.tile([128, 512], F32, tag="psO",
                                               name="pso")
                                ssl = slice(ssub * 128, (ssub + 1) * 128)
                                for dht in range(16):
                                    nc.tensor.matmul(pso[:],
                                                     csl[dht][:, ssl],
                                                     wos[dht][:],
                                                     start=(dht == 0),
                                                     stop=(dht == 15))
                                osb = phDo.tile([128, 512], F16,
                                                tag="osb", name="osb")
                                nc.vector.tensor_copy(osb[:], pso[:])
                                nc.sync.dma_start(out=out_sl[ssl, osl],
                                                  in_=osb[:])

    nc.compile()
    return nc


def _host_prep(inputs):
    """Build per-core input maps from the full problem inputs."""
    x = np.asarray(inputs["x"], np.float32)
    xT = np.ascontiguousarray(x.reshape(BS, HID).T)            # [HID, BS]
    wdkvT = np.asarray(inputs["d_kv_w"], np.float32).T         # [HID, C]
    wdqT = np.asarray(inputs["d_q_w"], np.float32).T
    bdkv_h = np.asarray(inputs["d_kv_b"], np.float32).reshape(4, 128, 1)
    bdq_h = np.asarray(inputs["d_q_b"], np.float32).reshape(4, 128, 1)

    wkv_in = np.ascontiguousarray(
        wdkvT.reshape(4, 4, 128, C).transpose(0, 2, 1, 3)).astype(np.float16)
    wdq_in = np.ascontiguousarray(
        wdqT.reshape(4, 4, 128, C).transpose(0, 2, 1, 3)).astype(np.float16)

    uk3 = np.asarray(inputs["u_k_w"], np.float32).reshape(H, Dh, C)
    uq3 = np.asarray(inputs["u_q_w"], np.float32).reshape(H, Dh, C)
    uv3 = np.asarray(inputs["u_v_w"], np.float32).reshape(H, Dh, C)
    buk2 = np.asarray(inputs["u_k_b"], np.float32).reshape(H, Dh)
    buq2 = np.asarray(inputs["u_q_b"], np.float32).reshape(H, Dh)
    qr3 = np.asarray(inputs["qr_w"], np.float32).reshape(H, Dr, C)

    # rope tables (positions 0..S-1)
    i32 = np.arange(32, dtype=np.float32)
    inv_freq = (10000.0 ** (-(2.0 * i32) / Dr)).astype(np.float32)  # [32]
    pos = np.arange(S, dtype=np.float32)
    ang = pos[None, :] * inv_freq[:, None]                     # [32, S]
    cos, sin = np.cos(ang), np.sin(ang)
    c1 = np.concatenate([cos, sin, cos, sin], 0).astype(np.float16)
    c2 = np.concatenate([-sin, cos, -sin, cos], 0).astype(np.float16)

    woT = np.ascontiguousarray(
        np.asarray(inputs["out_w"], np.float32).T.astype(np.float16))

    in_maps = []
    for j in range(NCORES):
        hs = [2 * j, 2 * j + 1]
        xT_l = xT[:, j * S_LOC:(j + 1) * S_LOC]
        xt_in = np.ascontiguousarray(
            xT_l.reshape(4, 4, 128, S_LOC).transpose(0, 2, 1, 3)
        ).astype(np.float16)

        ukT_l = uk3[hs].transpose(2, 0, 1).reshape(C, 256)
        uqT_l = uq3[hs].transpose(2, 0, 1).reshape(C, 256)
        uvT_l = uv3[hs].transpose(2, 0, 1).reshape(C, 256)
        we = [qr3[h, 0::2, :] for h in hs]    # [32, C] each
        wo_ = [qr3[h, 1::2, :] for h in hs]
        wrA = np.concatenate([we[0], we[0], we[1], we[1]], 0).T  # [C, 128]
        wrB = np.concatenate([wo_[0], wo_[0], wo_[1], wo_[1]], 0).T

        in_maps.append({
            "xt_in": xt_in, "wkv_in": wkv_in, "wdq_in": wdq_in,
            "bdkv": bdkv_h, "bdq": bdq_h,
            "ukT": np.ascontiguousarray(ukT_l.astype(np.float16)),
            "uqT": np.ascontiguousarray(uqT_l.astype(np.float16)),
            "uvT": np.ascontiguousarray(uvT_l.astype(np.float16)),
            "wraT": np.ascontiguousarray(wrA.astype(np.float16)),
            "wrbT": np.ascontiguousarray(wrB.astype(np.float16)),
            "buk": buk2[hs].reshape(2, 128, 1).copy(),
            "buq": buq2[hs].reshape(2, 128, 1).copy(),
            "c1": c1, "c2": c2,
            "woT": woT,
        })
    return in_maps


def kernel(**inputs):
    if "nc" not in _CACHE:
        _CACHE["nc"] = _build_program()
    nc = _CACHE["nc"]
    in_maps = _host_prep(inputs)
    res = run_bass_kernel_spmd(nc, in_maps, list(range(NCORES)))
    out = np.concatenate(
        [np.asarray(res.results[j]["out_slice"], np.float32)
         for j in range(NCORES)], 0)
    out = out + np.asarray(inputs["out_b"], np.float32)[None, :]
    return out.reshape(B, S, HID)


# revision 28
# speedup vs baseline: 1.3246x; 1.3246x over previous
"""Multi-Head Latent Attention (MLA) Bass kernel for 8 Trainium2 NeuronCores.

Sharding (v5):
  - latent projections (d_kv, d_q): sequence-sharded (BS/8 rows per core),
    fp16 AllGathers replicate the latents.
  - up-projections (u_k, u_v, u_q, qr) + attention: head-sharded, 2 heads/core.
  - context: AllToAll re-shards to sequence; out_proj sequence-parallel.

Precision strategy (error budget is the binding constraint; measured fp8e4m3
rms rel err is 5.8% and softmax amplifies score noise ~3.5x into the output,
so each fp8 stage costs ~1.5-2.4% of output rel err vs the 2e-2 gate):
  - Exactly ONE fp8 stage: the stored k8/q8 score operands.  The S*S score
    matmuls run fp8 MatmulPerfMode.DoubleRow at 0.5 cycles/row (4x bf16).
    Layout per head: [128 partitions, 2 k-tiles, BS]; tile0 = k_c dims
    0..127, tile1 = [rope 0..63 | zeros].  The dead zero rows are free --
    DoubleRow cost depends only on the moving free size.  Zero-padding is
    memset once (fp8 garbage could be NaN and NaN*0 propagates).
  - Everything else (latents, all projection weights, V, probs, out-proj)
    is fp16: same PE speed as bf16, 8x less quantization noise.
  - The softmax scale rides the exp activation's `scale` operand.
  - exp is batched 2 sk-tiles per activation ([128, 2048]); the Activation
    engine is the attention-phase bottleneck (~15.1us/block), PE runs under.
  - rope/V biases are zeros by problem spec and not injected on device where
    they would cost extra instructions (qr_b, u_v_b); out_b is added on host.
    d_kv_b/d_q_b/u_k_b/u_q_b ride along free on the copy path.
  - woT (out-proj weights) prefetch into SBUF during attention so phase D
    starts immediately after the AllToAll.  Write-only filler matmuls keep
    the PE clock (HAM p-state) warm across the two exposed collective waits.
"""
import sys
import os

for _p in ("/opt/trn_rl_repo", "/root/.axon_site/_ro/trn_rl_repo"):
    if os.path.isdir(_p) and _p not in sys.path:
        sys.path.insert(0, _p)

import math
import numpy as np
import ml_dtypes

F8NP = ml_dtypes.float8_e4m3

import concourse.bacc as bacc
import concourse.mybir as mybir
from concourse import tile
from concourse.bass_utils import run_bass_kernel_spmd
from concourse import bass_isa

# problem dims (hardcoded)
B, S, H, Dh, Dr, HID, C = 2, 2048, 16, 128, 64, 2048, 512
BS = B * S                      # 4096
NCORES = 8
H_LOC = H // NCORES             # 2
S_LOC = BS // NCORES            # 512
SCALE = 1.0 / math.sqrt(Dh + Dr)

F32 = mybir.dt.float32
F16 = mybir.dt.float16
F8 = mybir.dt.float8e4
DR = mybir.MatmulPerfMode.DoubleRow

_CACHE = {}


def _build_program():
    nc = bacc.Bacc("TRN2", target_bir_lowering=False, debug=False,
                   num_devices=NCORES)

    # ---- phase A inputs ----
    xt_in = nc.dram_tensor("xt_in", [4, 128, 4, S_LOC], F16, kind="ExternalInput")
    wkv_in = nc.dram_tensor("wkv_in", [4, 128, 4, C], F16, kind="ExternalInput")
    wdq_in = nc.dram_tensor("wdq_in", [4, 128, 4, C], F16, kind="ExternalInput")
    bdkv = nc.dram_tensor("bdkv", [4, 128, 1], F32, kind="ExternalInput")
    bdq = nc.dram_tensor("bdq", [4, 128, 1], F32, kind="ExternalInput")
    # ---- up-projection weights (fp16) ----
    ukT = nc.dram_tensor("ukT", [C, 256], F16, kind="ExternalInput")
    uqT = nc.dram_tensor("uqT", [C, 256], F16, kind="ExternalInput")
    uvT = nc.dram_tensor("uvT", [C, 256], F16, kind="ExternalInput")
    wraT = nc.dram_tensor("wraT", [C, 128], F16, kind="ExternalInput")
    wrbT = nc.dram_tensor("wrbT", [C, 128], F16, kind="ExternalInput")
    buk = nc.dram_tensor("buk", [2, 128, 1], F32, kind="ExternalInput")
    buq = nc.dram_tensor("buq", [2, 128, 1], F32, kind="ExternalInput")
    c1_in = nc.dram_tensor("c1", [128, S], F16, kind="ExternalInput")
    c2_in = nc.dram_tensor("c2", [128, S], F16, kind="ExternalInput")

    woT = nc.dram_tensor("woT", [H * Dh, HID], F16, kind="ExternalInput")
    out_sl = nc.dram_tensor("out_slice", [S_LOC, HID], F16, kind="ExternalOutput")

    groups = [list(range(NCORES))]
    EXP = mybir.ActivationFunctionType.Exp
    IDENT = mybir.ActivationFunctionType.Identity

    with tile.TileContext(nc) as tc:
        with tc.tile_pool(name="dram", bufs=1, space="DRAM") as dram:
            kv16_i = [dram.tile([2, 128, S_LOC], F16, name=f"kv16i{i}")
                      for i in range(2)]
            kv16_o = [dram.tile([NCORES, 2, 128, S_LOC], F16,
                                addr_space="Shared", name=f"kv16o{i}")
                      for i in range(2)]
            ql16_i = [dram.tile([2, 128, S_LOC], F16, name=f"ql16i{i}")
                      for i in range(2)]
            ql16_o = [dram.tile([NCORES, 2, 128, S_LOC], F16,
                                addr_space="Shared", name=f"ql16o{i}")
                      for i in range(2)]
            a2a_i = dram.tile([NCORES, H_LOC * Dh, S_LOC], F16)
            a2a_o = dram.tile([NCORES, H_LOC * Dh, S_LOC], F16, name="a2ao")

            with tc.tile_pool(name="const", bufs=1) as const:
                ones_col = const.tile([128, 1], F16)
                nc.vector.memset(ones_col[:], 1.0)

                # ====== persistent attention operand tiles ======
                with tc.tile_pool(name="attn", bufs=1) as attn:
                    # [128 part, 2 k-tiles, BS] fp8 DoubleRow score operands
                    k8 = [attn.tile([128, 2, BS], F8, tag=f"k8_{h}",
                                    name=f"k8_{h}") for h in range(2)]
                    q8 = [attn.tile([128, 2, BS], F8, tag=f"q8_{h}",
                                    name=f"q8_{h}") for h in range(2)]
                    # zero the dead rows of k-tile 1 once (fp8 garbage could
                    # be NaN, and NaN*0 propagates through the PE)
                    for t_ in (k8[0], k8[1], q8[0], q8[1]):
                        nc.vector.memset(t_[64:128, 1, :], 0.0)
                    v_sb = attn.tile([128, 32, 256], F16, tag="v", name="v_sb")
                    c1s = attn.tile([128, S], F16, tag="c1", name="c1s")
                    c2s = attn.tile([128, S], F16, tag="c2", name="c2s")

                    # resident operand for HAM-warming filler matmuls
                    junk8 = attn.tile([128, 2, S_LOC], F8, tag="junk8",
                                      name="junk8")
                    nc.vector.memset(junk8[:], 0.25)

                    # ============ Phase A: latents (sequence-sharded) ========
                    with tc.tile_pool(name="phA", bufs=1) as phA, \
                         tc.tile_pool(name="phAb", bufs=4) as phAb, \
                         tc.tile_pool(name="psA", bufs=2, space="PSUM") as psA:
                        engs = (nc.sync, nc.gpsimd, nc.scalar)
                        ei = 0
                        xts, wkv, wdq = [], [], []
                        for q4 in range(4):
                            xt = phA.tile([128, 4, S_LOC], F16, tag=f"xt{q4}",
                                          name=f"xt{q4}")
                            for i2 in range(2):
                                engs[ei % 3].dma_start(
                                    out=xt[:, 2 * i2:2 * i2 + 2, :],
                                    in_=xt_in[q4, :, 2 * i2:2 * i2 + 2, :])
                                ei += 1
                            xts.append(xt)
                            w = phA.tile([128, 4, C], F16, tag=f"wk{q4}",
                                         name=f"wk{q4}")
                            for i2 in range(2):
                                engs[ei % 3].dma_start(
                                    out=w[:, 2 * i2:2 * i2 + 2, :],
                                    in_=wkv_in[q4, :, 2 * i2:2 * i2 + 2, :])
                                ei += 1
                            wkv.append(w)
                        for q4 in range(4):
                            w = phA.tile([128, 4, C], F16, tag=f"wq{q4}",
                                         name=f"wq{q4}")
                            for i2 in range(2):
                                engs[ei % 3].dma_start(
                                    out=w[:, 2 * i2:2 * i2 + 2, :],
                                    in_=wdq_in[q4, :, 2 * i2:2 * i2 + 2, :])
                                ei += 1
                            wdq.append(w)

                        # ramp the PE clock while the startup DMAs land
                        psw = psA.tile([128, S_LOC], F32, tag="warm",
                                       name="psw")
                        for _ in range(60):
                            nc.tensor.matmul(psw[:], junk8[:, :, 0:128],
                                             junk8[:], start=True, stop=True,
                                             perf_mode=DR)
                        for li, (wfull, bdram, agi, ago) in enumerate(
                                ((wkv, bdkv, kv16_i, kv16_o),
                                 (wdq, bdq, ql16_i, ql16_o))):
                            for ct in range(4):
                                ps = psA.tile([128, S_LOC], F32, tag="ps",
                                              name="psa")
                                for ht in range(16):
                                    nc.tensor.matmul(
                                        ps[:],
                                        wfull[ht // 4][:, ht % 4,
                                                       ct * 128:(ct + 1) * 128],
                                        xts[ht // 4][:, ht % 4, :],
                                        start=(ht == 0), stop=(ht == 15))
                                bt = phAb.tile([128, 1], F32, tag="blat",
                                               name="blat")
                                nc.scalar.dma_start(out=bt[:], in_=bdram[ct])
                                l16 = phAb.tile([128, S_LOC], F16, tag="l16",
                                                bufs=3, name="l16")
                                nc.scalar.activation(l16[:], ps[:], IDENT,
                                                     bias=bt[:])
                                nc.sync.dma_start(out=agi[ct // 2][ct % 2],
                                                  in_=l16[:])
                                if ct % 2 == 1:
                                    nc.gpsimd.collective_compute(
                                        "AllGather", mybir.AluOpType.bypass,
                                        replica_groups=groups,
                                        ins=[agi[ct // 2].opt()],
                                        outs=[ago[ct // 2].opt()])

                    # rope tables (issued after the phase-A critical DMAs)
                    nc.sync.dma_start(out=c1s[:], in_=c1_in[:])
                    nc.sync.dma_start(out=c2s[:], in_=c2_in[:])
                    # persistent up-projection weights
                    upw = {}
                    for nm, t, w_ in (("uk", ukT, 256), ("uq", uqT, 256),
                                      ("uv", uvT, 256),
                                      ("wra", wraT, 128), ("wrb", wrbT, 128)):
                        tl = []
                        for ct in range(4):
                            wt = attn.tile([128, w_], F16, tag=f"{nm}{ct}",
                                           name=f"{nm}{ct}")
                            nc.sync.dma_start(
                                out=wt[:], in_=t[ct * 128:(ct + 1) * 128, :])
                            tl.append(wt)
                        upw[nm] = tl
                    bias_t = {}
                    for nm, t in (("buk", buk), ("buq", buq)):
                        tl = []
                        for h in range(2):
                            bt_ = attn.tile([128, 1], F32, tag=f"{nm}{h}",
                                            name=f"{nm}{h}")
                            nc.scalar.dma_start(out=bt_[:], in_=t[h])
                            tl.append(bt_)
                        bias_t[nm] = tl
                    # woT prefetch: wo_sb[:, dht, ot, :] = woT[dht*128+p,
                    # ot*512+c] for ot 0..1; second column-half streams in
                    # phase D.  Half loads now, half during attention.
                    wo_sb = attn.tile([128, 16, 2, S_LOC], F16, tag="wo",
                                      name="wo_sb")

                    # keep the PE clock un-throttled across the AG-kv wait:
                    # write-only filler matmuls on resident tiles.
                    with tc.tile_pool(name="psj", bufs=1, space="PSUM") as psjp:
                        psj = psjp.tile([128, S_LOC], F32, tag="junk",
                                        name="psj")
                        for _ in range(30):
                            nc.tensor.matmul(psj[:], junk8[:, :, 0:128],
                                             junk8[:], start=True, stop=True,
                                             perf_mode=DR)

                    # ========= phase B: kv-dependent (k_c, rope-k, V) ========
                    def up_project(j2, w_lo, b_lo, dst, src, phX, psX):
                        """fp16 up-projection of one 512-col block into the
                        fp8 [128, 2, BS] DoubleRow score layout."""
                        sl = slice(j2 * 512, (j2 + 1) * 512)
                        pos = slice((j2 % 4) * 512, (j2 % 4) * 512 + 512)
                        for h in range(2):
                            psK = psX.tile([128, 512], F32, tag="psK",
                                           bufs=2, name="psk")
                            hc = slice(h * 128, (h + 1) * 128)
                            for ct in range(4):
                                nc.tensor.matmul(psK[:],
                                                 w_lo[ct][:, hc],
                                                 src[ct][:],
                                                 start=(ct == 0),
                                                 stop=(ct == 3))
                            nc.scalar.activation(dst[h][:, 0, sl],
                                                 psK[:], IDENT,
                                                 bias=b_lo[h][:])
                        # rope for both heads stacked on 128 partitions
                        psRA = psX.tile([128, 512], F32, tag="psRA", bufs=2,
                                        name="psra")
                        psRB = psX.tile([128, 512], F32, tag="psRB", bufs=2,
                                        name="psrb")
                        for ct in range(4):
                            nc.tensor.matmul(psRA[:], upw["wra"][ct][:],
                                             src[ct][:],
                                             start=(ct == 0), stop=(ct == 3))
                        for ct in range(4):
                            nc.tensor.matmul(psRB[:], upw["wrb"][ct][:],
                                             src[ct][:],
                                             start=(ct == 0), stop=(ct == 3))
                        t1 = phX.tile([128, 512], F16, tag="t1", bufs=2,
                                      name="t1")
                        nc.vector.tensor_mul(t1[:], psRA[:], c1s[:, pos])
                        t2 = phX.tile([128, 512], F16, tag="t2", bufs=2,
                                      name="t2")
                        nc.vector.tensor_mul(t2[:], psRB[:], c2s[:, pos])
                        stgr = phX.tile([128, 512], F8, tag="stgr", bufs=2,
                                        name="stgr")
                        nc.vector.tensor_add(stgr[:], t1[:], t2[:])
                        # partition-shift the per-head rope halves into place
                        nc.sync.dma_start(out=dst[0][0:64, 1, sl],
                                          in_=stgr[0:64, :])
                        nc.sync.dma_start(out=dst[1][0:64, 1, sl],
                                          in_=stgr[64:128, :])

                    with tc.tile_pool(name="phB", bufs=1) as phB, \
                         tc.tile_pool(name="psB", bufs=1, space="PSUM") as psB:
                        for j2 in range(8):
                            kv_sb = []
                            for ct in range(4):
                                kt_ = phB.tile([128, 512], F16,
                                               tag=f"kv_{ct}", bufs=3,
                                               name=f"kv_{ct}")
                                eng = (nc.scalar, nc.gpsimd, nc.sync,
                                       nc.scalar)[ct]
                                eng.dma_start(
                                    out=kt_[:],
                                    in_=kv16_o[ct // 2][j2, ct % 2])
                                kv_sb.append(kt_)
                            up_project(j2, upw["uk"], bias_t["buk"],
                                       k8, kv_sb, phB, psB)
                            # V projection (fp16); u_v_b is zero by problem
                            # spec (bias varies along the free dim here, so
                            # not fusable into the copy)
                            for ss in range(4):
                                psv_ = psB.tile([128, 256], F32, tag="psV",
                                                bufs=2, name="psv")
                                ssl = slice(ss * 128, (ss + 1) * 128)
                                for ct in range(4):
                                    nc.tensor.matmul(psv_[:],
                                                     kv_sb[ct][:, ssl],
                                                     upw["uv"][ct][:],
                                                     start=(ct == 0),
                                                     stop=(ct == 3))
                                st = j2 * 4 + ss
                                nc.scalar.activation(v_sb[:, st, :], psv_[:],
                                                     IDENT)

                    # preload the gpsimd extended-instruction lib before
                    # attention so the first partition_broadcast doesn't pay
                    # the LOAD_LIB swap mid-attention.
                    pbin = const.tile([1, 64], F32)
                    nc.vector.memset(pbin[:], 1.0)
                    pbout = const.tile([128, 64], F32)
                    nc.gpsimd.partition_broadcast(pbout[:], pbin[:])

                    # ================= pass 2: q-side projections ============
                    with tc.tile_pool(name="phQ", bufs=1) as phQ, \
                         tc.tile_pool(name="psQ", bufs=1, space="PSUM") as psQ:
                        for g in range(8):
                            ql_sb = []
                            for ct in range(4):
                                qt_ = phQ.tile([128, 512], F16,
                                               tag=f"ql_{ct}", bufs=3,
                                               name=f"ql_{ct}")
                                eng = (nc.scalar, nc.gpsimd, nc.sync,
                                       nc.scalar)[ct]
                                eng.dma_start(
                                    out=qt_[:],
                                    in_=ql16_o[ct // 2][g, ct % 2])
                                ql_sb.append(qt_)
                            up_project(g, upw["uq"], bias_t["buq"],
                                       q8, ql_sb, phQ, psQ)
                        # first half of woT while attention operands settle
                        for dht in range(8):
                            nc.gpsimd.dma_start(
                                out=wo_sb[:, dht, :, :],
                                in_=woT[dht * 128:(dht + 1) * 128, 0:1024])

                    # ================= Phase C: attention ====================
                    with tc.tile_pool(name="phC", bufs=1) as phC, \
                         tc.tile_pool(name="psC", bufs=1, space="PSUM") as psC:
                        psvs = [psC.tile([128, 512], F32, tag=f"psv{h}",
                                         bufs=1, name=f"psv{h}")
                                for h in range(2)]

                        def denom_finish(p1):
                            """cross-partition sum (2 PE colsum matmuls),
                            1-lane reciprocal, gpsimd broadcast.  The chain
                            runs early in the next block, off the PE's
                            steady-state score/PV stream."""
                            dsum = psC.tile([1, 2, 512], F32, tag="dsum",
                                            bufs=1, name="dsum")
                            for h in range(2):
                                nc.tensor.matmul(dsum[0:1, h, :], ones_col[:],
                                                 p1[:, h, :],
                                                 start=True, stop=True)
                            rec = phC.tile([1, 2, 512], F32, tag="rec",
                                           bufs=1, name="rec")
                            recb = phC.tile([128, 2, 512], F32, tag="recb",
                                            bufs=1, name="recb")
                            nc.vector.reciprocal_approx_accurate(
                                out=rec[0:1, :, :], in_=dsum[0:1, :, :],
                                scratch=recb[0:1, :, :])
                            nc.gpsimd.partition_broadcast(recb[:],
                                                          rec[0:1, :, :])
                            return recb

                        def finish_stg(pg, recb):
                            for h in range(2):
                                stg = phC.tile([128, 512], F16, tag=f"stg{h}",
                                               bufs=1, name=f"stg{h}")
                                nc.vector.tensor_mul(stg[:], psvs[h][:],
                                                     recb[:, h, :])
                                nc.sync.dma_start(
                                    out=a2a_i[pg, h * 128:(h + 1) * 128, :],
                                    in_=stg[:])

                        # software pipeline: block g's score/exp stream carries
                        # block g-1's PV, denominator chain and normalization.
                        prev = None
                        for g in range(8):
                            b, sqb = g // 4, g % 4
                            qsl = slice(b * S + sqb * 512,
                                        b * S + sqb * 512 + 512)
                            probs = phC.tile([128, 16, 2, 512], F16,
                                             tag="probs", bufs=2, name="probs")
                            # per-partition denominator partials accumulate on
                            # DVE as each exp retires (fp16; the 128-way
                            # cross-partition sum happens in fp32 on the PE)
                            p1 = phC.tile([128, 2, 512], F16, tag="p1",
                                          bufs=2, name="p1")
                            if prev is not None:
                                pg, pb, probs_p, p1_p = prev
                                recb = None
                                pv_idx = 0
                            for skt in range(16):
                                psb_ = psC.tile([128, 2, 512], F32,
                                                tag="ps2", bufs=2, name="ps2")
                                ksl = slice(b * S + skt * 128,
                                            b * S + skt * 128 + 128)
                                for h in range(2):
                                    nc.tensor.matmul(
                                        psb_[:, h, :],
                                        k8[h][:, :, ksl],
                                        q8[h][:, :, qsl],
                                        start=True, stop=True,
                                        perf_mode=DR)
                                nc.scalar.activation(
                                    probs[:, skt, :, :],
                                    psb_[:], EXP, scale=SCALE)
                                if skt == 0:
                                    nc.vector.tensor_copy(
                                        p1[:], probs[:, 0, :, :])
                                else:
                                    nc.vector.tensor_add(
                                        p1[:], p1[:], probs[:, skt, :, :])
                                if prev is not None:
                                    # prev-block PV: 16 sk-tiles x 2 heads,
                                    # ~2-3 matmuls per score group
                                    while (pv_idx < 32 and
                                           pv_idx * 14 // 32 <= skt and
                                           skt < 14):
                                        sktp = pv_idx // 2
                                        h = pv_idx % 2
                                        nc.tensor.matmul(
                                            psvs[h][:],
                                            v_sb[:, pb * 16 + sktp,
                                                 h * 128:(h + 1) * 128],
                                            probs_p[:, sktp, h, :],
                                            start=(sktp == 0),
                                            stop=(sktp == 15))
                                        pv_idx += 1
                                    if skt == 2:
                                        recb = denom_finish(p1_p)
                            if prev is not None:
                                finish_stg(prev[0], recb)
                            # second half of woT, spread across blocks
                            nc.gpsimd.dma_start(
                                out=wo_sb[:, 8 + g, :, :],
                                in_=woT[(8 + g) * 128:(9 + g) * 128, 0:1024])
                            prev = (g, b, probs, p1)
                        # drain the last block
                        pg, pb, probs_p, p1_p = prev
                        for sktp in range(16):
                            for h in range(2):
                                nc.tensor.matmul(
                                    psvs[h][:],
                                    v_sb[:, pb * 16 + sktp,
                                         h * 128:(h + 1) * 128],
                                    probs_p[:, sktp, h, :],
                                    start=(sktp == 0), stop=(sktp == 15))
                        finish_stg(pg, denom_finish(p1_p))
                        nc.gpsimd.collective_compute(
                            "AllToAll", mybir.AluOpType.bypass,
                            replica_groups=groups,
                            ins=[a2a_i.opt()], outs=[a2a_o.opt()])

                    # ============== Phase D: out projection ==================
                    with tc.tile_pool(name="phD", bufs=1) as phD, \
                         tc.tile_pool(name="phDw", bufs=2) as phDw, \
                         tc.tile_pool(name="phDo", bufs=3) as phDo, \
                         tc.tile_pool(name="psD", bufs=2, space="PSUM") as psD:
                        # ot=2,3 weight columns load during the A2A wait
                        wos_hi = []
                        for dht in range(16):
                            wo = phDw.tile([128, 2, 512], F16,
                                           tag=f"wo{dht}", bufs=1,
                                           name=f"wo{dht}")
                            eng = (nc.sync, nc.gpsimd, nc.scalar)[dht % 3]
                            eng.dma_start(
                                out=wo[:],
                                in_=woT[dht * 128:(dht + 1) * 128,
                                        1024:2048])
                            wos_hi.append(wo)
                        # filler over the AllToAll wait
                        psj2 = psD.tile([128, 512], F32, tag="junk",
                                        bufs=1, name="psj2")
                        for _ in range(110):
                            nc.tensor.matmul(psj2[:], junk8[:, :, 0:128],
                                             junk8[:], start=True, stop=True,
                                             perf_mode=DR)
                        csl = []
                        for dht in range(16):
                            cf = phD.tile([128, S_LOC], F16,
                                          tag=f"cf{dht}", name=f"cf{dht}")
                            eng = nc.sync if dht % 2 == 0 else nc.gpsimd
                            eng.dma_start(
                                out=cf[:],
                                in_=a2a_o[dht // 2,
                                          (dht % 2) * 128:
                                          (dht % 2) * 128 + 128, :])
                            csl.append(cf)
                        for ot in range(4):
                            osl = slice(ot * 512, (ot + 1) * 512)
                            if ot < 2:
                                wos = [wo_sb[:, dht, ot, :]
                                       for dht in range(16)]
                            else:
                                wos = [wos_hi[dht][:, ot - 2, :]
                                       for dht in range(16)]
                            for ssub in range(4):
                                pso = psD.tile([128, 512], F32, tag="psO",
                                               bufs=4, name="pso")
                                ssl = slice(ssub * 128, (ssub + 1) * 128)
                                for dht in range(16):
                                    nc.tensor.matmul(pso[:],
                                                     csl[dht][:, ssl],
                                                     wos[dht][:],
                                                     start=(dht == 0),
                                                     stop=(dht == 15))
                                osb = phDo.tile([128, 512], F16,
                                                tag="osb", bufs=4,
                                                name="osb")
                                nc.vector.tensor_copy(osb[:], pso[:])
                                nc.sync.dma_start(out=out_sl[ssl, osl],
                                                  in_=osb[:])

    nc.compile()
    return nc


def _host_prep(inputs):
    """Build per-core input maps from the full problem inputs."""
    x = np.asarray(inputs["x"], np.float32)
    xT = np.ascontiguousarray(x.reshape(BS, HID).T)            # [HID, BS]
    wdkvT = np.asarray(inputs["d_kv_w"], np.float32).T         # [HID, C]
    wdqT = np.asarray(inputs["d_q_w"], np.float32).T
    bdkv_h = np.asarray(inputs["d_kv_b"], np.float32).reshape(4, 128, 1)
    bdq_h = np.asarray(inputs["d_q_b"], np.float32).reshape(4, 128, 1)

    wkv_in = np.ascontiguousarray(
        wdkvT.reshape(4, 4, 128, C).transpose(0, 2, 1, 3)).astype(np.float16)
    wdq_in = np.ascontiguousarray(
        wdqT.reshape(4, 4, 128, C).transpose(0, 2, 1, 3)).astype(np.float16)

    uk3 = np.asarray(inputs["u_k_w"], np.float32).reshape(H, Dh, C)
    uq3 = np.asarray(inputs["u_q_w"], np.float32).reshape(H, Dh, C)
    uv3 = np.asarray(inputs["u_v_w"], np.float32).reshape(H, Dh, C)
    buk2 = np.asarray(inputs["u_k_b"], np.float32).reshape(H, Dh)
    buq2 = np.asarray(inputs["u_q_b"], np.float32).reshape(H, Dh)
    qr3 = np.asarray(inputs["qr_w"], np.float32).reshape(H, Dr, C)

    # rope tables (positions 0..S-1)
    i32 = np.arange(32, dtype=np.float32)
    inv_freq = (10000.0 ** (-(2.0 * i32) / Dr)).astype(np.float32)  # [32]
    pos = np.arange(S, dtype=np.float32)
    ang = pos[None, :] * inv_freq[:, None]                     # [32, S]
    cos, sin = np.cos(ang), np.sin(ang)
    c1 = np.concatenate([cos, sin, cos, sin], 0).astype(np.float16)
    c2 = np.concatenate([-sin, cos, -sin, cos], 0).astype(np.float16)

    woT = np.ascontiguousarray(
        np.asarray(inputs["out_w"], np.float32).T.astype(np.float16))

    in_maps = []
    for j in range(NCORES):
        hs = [2 * j, 2 * j + 1]
        xT_l = xT[:, j * S_LOC:(j + 1) * S_LOC]
        xt_in = np.ascontiguousarray(
            xT_l.reshape(4, 4, 128, S_LOC).transpose(0, 2, 1, 3)
        ).astype(np.float16)

        ukT_l = uk3[hs].transpose(2, 0, 1).reshape(C, 256)
        uqT_l = uq3[hs].transpose(2, 0, 1).reshape(C, 256)
        uvT_l = uv3[hs].transpose(2, 0, 1).reshape(C, 256)
        we = [qr3[h, 0::2, :] for h in hs]    # [32, C] each
        wo_ = [qr3[h, 1::2, :] for h in hs]
        wrA = np.concatenate([we[0], we[0], we[1], we[1]], 0).T  # [C, 128]
        wrB = np.concatenate([wo_[0], wo_[0], wo_[1], wo_[1]], 0).T

        in_maps.append({
            "xt_in": xt_in, "wkv_in": wkv_in, "wdq_in": wdq_in,
            "bdkv": bdkv_h, "bdq": bdq_h,
            "ukT": np.ascontiguousarray(ukT_l.astype(np.float16)),
            "uqT": np.ascontiguousarray(uqT_l.astype(np.float16)),
            "uvT": np.ascontiguousarray(uvT_l.astype(np.float16)),
            "wraT": np.ascontiguousarray(wrA.astype(np.float16)),
            "wrbT": np.ascontiguousarray(wrB.astype(np.float16)),
            "buk": buk2[hs].reshape(2, 128, 1).copy(),
            "buq": buq2[hs].reshape(2, 128, 1).copy(),
            "c1": c1, "c2": c2,
            "woT": woT,
        })
    return in_maps


def kernel(**inputs):
    if "nc" not in _CACHE:
        _CACHE["nc"] = _build_program()
    nc = _CACHE["nc"]
    in_maps = _host_prep(inputs)
    res = run_bass_kernel_spmd(nc, in_maps, list(range(NCORES)))
    out = np.concatenate(
        [np.asarray(res.results[j]["out_slice"], np.float32)
         for j in range(NCORES)], 0)
    out = out + np.asarray(inputs["out_b"], np.float32)[None, :]
    return out.reshape(B, S, HID)
